# revision 29
# baseline (speedup 1.0000x reference)
"""Trainium2 Bass kernel for nn_DetectionLoss (SSD-style detection loss).

Data-parallel over the batch: 16 images, 8 NeuronCores, 2 images per core.

v2 design (vs the f32 baseline):
- The [A, G] match grid is computed in fp16 in a g-major layout
  [partition, gt, anchor-col] so every heavy DVE op runs in the 2x fp16
  mode with a packed last AP dim. Broadcast operands that vary along gt
  use width-4 replicated tiles (gt_rep4 / areaG_rep4 / cmb_rep4) so the
  last AP dim stays packed.
- t = inter / (areaA + areaG + eps) replaces IoU: iou = t/(1-t) is a
  monotone map, so row/col argmax and the iou>0.5 threshold (t > 1/3)
  are preserved while the union subtraction disappears.
- Engine split: Act does relu + reciprocal + PSUM->SBUF copies, Pool
  (gpsimd) does the segmented reduces, PE does transposes + the
  matched-gt one-hot matmul (8 cols: x1,y1,x2,y2,area,cx,cy,count).
  Division by the count column fixes fp16 is-max ties (averages tied
  gts, which have equal IoU to within fp16 resolution).
- The two images' grid supertiles are software-pipelined (stage A:
  elementwise through t; stage B: is-max + transpose/matmul) so the DVE
  never waits on the Pool row-max.
- DIoU + focal run batched over both images in planar fp16 layout;
  enclosing-box / center squares are scaled by 1/16 to stay in range.
- Hard-negative mining: 2-level x 16-bin threshold search (resolution
  256) with fp16 counting split across DVE and Pool; host applies the
  exact-sum + midpoint boundary-bin correction.
"""
import sys

sys.path.insert(0, '/opt/trn_rl_repo')

import numpy as np
import concourse.bass as bass
import concourse.bacc as bacc
import concourse.mybir as mybir
from concourse.tile import TileContext
from concourse.bass_utils import run_bass_kernel_spmd
from concourse.masks import make_identity
from contextlib import ExitStack

Alu = mybir.AluOpType
Act = mybir.ActivationFunctionType
Ax = mybir.AxisListType
F32 = mybir.dt.float32
F16 = mybir.dt.float16
I32 = mybir.dt.int32

P = 128
A = 65536
G = 32
IMG = 2            # images per core
NCORE = 8
COLS = A // P      # 512 anchor columns per partition
U = 64             # anchor columns per supertile
WG = U * G         # 2048 grid elems per supertile per partition
NSUP = COLS // U   # 8 supertiles
EPS = 1e-7
NBIN = 16          # histogram bins per mining level
NLEV = 2           # mining levels (resolution NBIN**NLEV = 256)
NEG_POS_RATIO = 3.0
LN_THIRD = -1.0986123  # iou > 0.5  <=>  ln(inter) - ln(S) > ln(1/3)
LNB = 6e-5             # ln bias: ln(x + LNB) keeps ln finite at x=0
CSC = 1.0 / 16.0   # coordinate scale before squaring in fp16 diou


def _build_nc():
    nc = bacc.Bacc("TRN2", target_bir_lowering=False, debug=False)
    anch_d = nc.dram_tensor("anch", [4, P, COLS], F16, kind="ExternalInput")
    bbox_d = nc.dram_tensor("bbox", [IMG, 4, P, COLS], F16, kind="ExternalInput")
    conf_d = nc.dram_tensor("conf", [IMG, P, COLS], F16, kind="ExternalInput")
    # gt coords k-major: [IMG, 1, 4*G]  ([k*G+g] = gt[g,k])
    gt_d = nc.dram_tensor("gtb", [IMG, 1, 4 * G], F16, kind="ExternalInput")
    # matched-gt matmul weights: rows r = g*4+ci, cols ci*8+v,
    # v in (x1,y1,x2,y2,area,cx,cy,1)
    gtm_d = nc.dram_tensor("gtm", [IMG, P, G], F16, kind="ExternalInput")
    res_d = nc.dram_tensor("res", [IMG, 1, 8], F32, kind="ExternalOutput")

    v = nc.vector
    sc = nc.scalar
    pe = nc.tensor
    gp = nc.gpsimd

    with TileContext(nc) as tc, ExitStack() as ctx:
        pool = ctx.enter_context(tc.tile_pool(name="main", bufs=1))
        pspool = ctx.enter_context(tc.tile_pool(name="ps", bufs=1, space="PSUM"))

        def T(name, cols, parts=P, dt=F16):
            return pool.tile([parts, cols], dt, name=name)

        def TF(name, cols, parts=P):
            return pool.tile([parts, cols], F32, name=name)

        # ---------------- persistent tiles ----------------
        anch = T("anch", 4 * COLS)              # planar [k][c]
        areaA = T("areaA", COLS)
        bbox = T("bbox", 4 * IMG * COLS)        # [k][img][c] planar
        conf = T("conf", IMG * COLS)            # [img][c]
        grid = [T(f"grid{b}", G * COLS) for b in range(IMG)]
        rowmax = [T(f"rowmax{b}", COLS) for b in range(IMG)]
        matched2 = T("matched2", 8 * IMG * COLS)    # planes [v][i][c]
        colacc = [T(f"colacc{b}", G) for b in range(IMG)]
        colaccW = [T(f"colaccW{b}", G * U) for b in range(IMG)]
        forcedc = [T(f"forcedc{b}", COLS) for b in range(IMG)]
        pos2 = T("pos2", IMG * COLS)            # [img][c]
        nv2 = T("nv2", IMG * COLS)              # [img][c]
        gtall = [T(f"gtall{b}", 4 * G) for b in range(IMG)]   # [k][g]
        gtmat = [T(f"gtmat{b}", G) for b in range(IMG)]       # [128, 32]
        gt_rep4 = [T(f"gt_rep4{b}", 4 * G * 4) for b in range(IMG)]  # [kg][4]
        areaG4 = [T(f"areaG4{b}", G * 4) for b in range(IMG)]        # [g][4]
        cmb4 = [T(f"cmb4{b}", G * 4) for b in range(IMG)]            # [g][4]
        sG = [T(f"sG{b}", G) for b in range(IMG)]

        # stage-A scratch (per image)
        lt = [T(f"lt{i}", 2 * WG) for i in range(IMG)]
        rb = [T(f"rb{i}", 2 * WG) for i in range(IMG)]
        inter = [T(f"inter{i}", WG) for i in range(IMG)]
        ssum = [T(f"ssum{i}", WG) for i in range(IMG)]
        # stage-B scratch (per image)
        ismax = [T(f"ismax{i}", WG) for i in range(IMG)]
        tsb = [T(f"tsb{i}", 4 * P) for i in range(4)]   # [img*2 + q%2]
        rfold = [T(f"rfold{i}", G * U // 2) for i in range(IMG)]

        # diou/focal scratch (batched over img)
        w0 = T("w0", 2 * IMG * COLS)
        w1_ = T("w1_", 2 * IMG * COLS)
        s0 = T("s0", IMG * COLS)
        s1 = T("s1", IMG * COLS)
        s2 = T("s2", IMG * COLS)
        s3 = T("s3", IMG * COLS)
        s4 = [T(f"s4_{b}", COLS) for b in range(IMG)]
        s5 = [T(f"s5_{b}", COLS) for b in range(IMG)]

        ident = T("ident", P)                   # fp16 identity
        ident32 = TF("ident32", P)
        ones_col = TF("ones_col", 1)
        ones_row = TF("ones_row", P, parts=1)
        ones_row16 = T("ones_row16", P, parts=1)
        colT = [T(f"colT{b}", P, parts=G) for b in range(IMG)]
        cmax_col = [T(f"cmax_col{b}", 1, parts=G) for b in range(IMG)]
        cm_row = [T(f"cm_row{b}", G, parts=1) for b in range(IMG)]
        cmb = [T(f"cmb{b}", G) for b in range(IMG)]
        mx_row = [TF(f"mx_row{b}", P, parts=1) for b in range(IMG)]
        npp = [TF(f"npp{b}", 1) for b in range(IMG)]
        locsum_pp = [TF(f"locsum_pp{b}", 1) for b in range(IMG)]
        possum_pp = [TF(f"possum_pp{b}", 1) for b in range(IMG)]
        cnt_pp = [TF(f"cnt_pp{b}", 1) for b in range(IMG)]
        sum_pp = [TF(f"sum_pp{b}", 1) for b in range(IMG)]
        maxv_pp = [TF(f"maxv_pp{b}", 1) for b in range(IMG)]
        maxvb = [TF(f"maxvb{b}", 1) for b in range(IMG)]
        w1c = [TF(f"w1c{b}", 1) for b in range(IMG)]
        tau_b = [TF(f"tau_b{b}", 1) for b in range(IMG)]
        stack = [TF(f"stack{b}", 4) for b in range(IMG)]
        iota_f = TF("iota_f", NBIN)
        thr = [TF(f"thr{b}", NBIN) for b in range(IMG)]
        cge = [TF(f"cge{b}", NBIN) for b in range(IMG)]
        sink = [T(f"sink{b}", COLS) for b in range(IMG)]
        wl = [[TF(f"wl{b}_{l}", 1) for l in range(NLEV)] for b in range(IMG)]
        lo_b = [[TF(f"lo_b{b}_{l}", 1) for l in range(NLEV)] for b in range(IMG)]
        cget = [TF(f"cget{b}", NBIN, parts=1) for b in range(IMG)]
        gek = [TF(f"gek{b}", NBIN, parts=1) for b in range(IMG)]
        scnt = [TF(f"scnt{b}", 1, parts=1) for b in range(IMG)]
        lo_new = [TF(f"lo_new{b}", 1, parts=1) for b in range(IMG)]
        tau = [[TF(f"tau{b}_{l}", 1, parts=1) for l in range(NLEV)]
               for b in range(IMG)]
        maxv1 = [TF(f"maxv1{b}", 1, parts=1) for b in range(IMG)]
        npos1 = [TF(f"npos1{b}", 1, parts=1) for b in range(IMG)]
        k1 = [TF(f"k1{b}", 1, parts=1) for b in range(IMG)]
        k2 = [TF(f"k2{b}", 1, parts=1) for b in range(IMG)]
        kk = [TF(f"kk{b}", 1, parts=1) for b in range(IMG)]
        kk2 = TF("kk2", 1, parts=1)
        thrn = [TF(f"thrn{b}", NBIN) for b in range(IMG)]
        res_sb = [TF(f"res_sb{b}", 8, parts=1) for b in range(IMG)]
        iota_i = pool.tile([P, NBIN], I32, name="iota_i")

        # ---------------- constants & loads ----------------
        anchsb = anch[:].rearrange("p (k c) -> p k c", c=COLS)
        for k in range(4):
            nc.sync.dma_start(anchsb[:, k, :], anch_d[k])
        bbsb = bbox[:].rearrange("p (k i c) -> p k i c", i=IMG, c=COLS)
        cfsb = conf[:].rearrange("p (i c) -> p i c", c=COLS)
        for b in range(IMG):
            for k in range(4):
                nc.sync.dma_start(bbsb[:, k, b, :], bbox_d[b, k])
            nc.sync.dma_start(cfsb[:, b, :], conf_d[b])
            nc.sync.dma_start(gtall[b][:],
                              gt_d[b].squeeze(0).partition_broadcast(P))
            nc.sync.dma_start(gtmat[b][:], gtm_d[b])
        lnb_c = TF("lnb_c", 1)
        v.memset(lnb_c[:], float(LNB))
        v.memset(ones_col[:], 1.0)
        v.memset(ones_row[:], 1.0)
        v.memset(ones_row16[:], 1.0)
        make_identity(nc, ident[:])
        make_identity(nc, ident32[:])
        nc.gpsimd.iota(iota_i[:], pattern=[[1, NBIN]], base=0, channel_multiplier=0)
        v.tensor_copy(iota_f[:], iota_i[:])

        anch3 = anch[:].rearrange("p (k c) -> p k c", c=COLS)
        v.tensor_tensor(out=s4[0][:], in0=anch3[:, 2, :], in1=anch3[:, 0, :],
                        op=Alu.subtract)
        v.tensor_tensor(out=s5[0][:], in0=anch3[:, 3, :], in1=anch3[:, 1, :],
                        op=Alu.subtract)
        v.tensor_tensor(out=areaA[:], in0=s4[0][:], in1=s5[0][:], op=Alu.mult)

        def pbcast(dst, src_row, n=1, f16=False):
            """Broadcast a [1, n] partition-0 row to [P, n] via a K=1 matmul."""
            bc_ps = pspool.tile([P, G], F32, name="bc_ps", tag="pss")
            orow = ones_row16 if f16 else ones_row
            nc.tensor.matmul(bc_ps[:, 0:n], orow[:], src_row)
            v.tensor_copy(dst, bc_ps[:, 0:n])

        def expand4(dst, src, n):
            """dst[p, n*4] <- src[p, n] replicated 4x along a new inner dim."""
            d3 = dst.rearrange("p (n r) -> p n r", r=4)
            v.tensor_copy(d3, src.unsqueeze(2).to_broadcast([P, n, 4]))

        # ---------------- per-image gt setup ----------------
        for b in range(IMG):
            ga = gtall[b][:].rearrange("p (k g) -> p k g", g=G)
            v.tensor_tensor(out=s4[b][:, 0:G], in0=ga[:, 2, :], in1=ga[:, 0, :],
                            op=Alu.subtract)
            v.tensor_tensor(out=s5[b][:, 0:G], in0=ga[:, 3, :], in1=ga[:, 1, :],
                            op=Alu.subtract)
            v.tensor_tensor(out=sG[b][:], in0=s4[b][:, 0:G], in1=s5[b][:, 0:G],
                            op=Alu.mult)
            expand4(gt_rep4[b][:], gtall[b][:], 4 * G)
            expand4(areaG4[b][:], sG[b][:], G)

        # ---------------- grid phase (pipelined over both images) ----------
        def grid_stage_a(b, s):
            csl = slice(s * U, (s + 1) * U)
            lt4 = lt[b][:].rearrange("p (kg c) -> p kg c", c=U)
            rb4 = rb[b][:].rearrange("p (kg c) -> p kg c", c=U)
            a_lo = anch3[:, 0:2, csl].unsqueeze(2).to_broadcast([P, 2, G, U])
            a_hi = anch3[:, 2:4, csl].unsqueeze(2).to_broadcast([P, 2, G, U])
            gr = gt_rep4[b][:].rearrange("p (kg r) -> p kg r", r=4)
            g_lo = gr[:, 0:2 * G, :].unsqueeze(2) \
                .to_broadcast([P, 2 * G, U // 4, 4])
            g_hi = gr[:, 2 * G:4 * G, :].unsqueeze(2) \
                .to_broadcast([P, 2 * G, U // 4, 4])
            v.tensor_tensor(out=lt4, in0=a_lo, in1=g_lo, op=Alu.max)
            v.tensor_tensor(out=rb4, in0=a_hi, in1=g_hi, op=Alu.min)
            v.tensor_tensor(out=lt[b][:], in0=rb[b][:], in1=lt[b][:],
                            op=Alu.subtract)
            sc.activation(rb[b][:], lt[b][:], Act.Relu)   # wh
            wh4 = rb[b][:].rearrange("p (k gc) -> p k gc", k=2)
            gp.tensor_tensor(out=inter[b][:], in0=wh4[:, 0, :], in1=wh4[:, 1, :],
                             op=Alu.mult)

        def grid_s_add(b, s):
            csl = slice(s * U, (s + 1) * U)
            aA = areaA[:, csl].unsqueeze(1).to_broadcast([P, G, U])
            aG = areaG4[b][:].rearrange("p (g r) -> p g r", r=4) \
                .unsqueeze(2).to_broadcast([P, G, U // 4, 4])
            seng = v if b == 0 else gp
            seng.tensor_tensor(out=ssum[b][:], in0=aA, in1=aG, op=Alu.add)

        def grid_lns(b, s):
            sc.activation(inter[b][:], inter[b][:], Act.Ln, bias=lnb_c[:])
            sc.activation(ssum[b][:], ssum[b][:], Act.Ln)

        def grid_stage_a2(b, s):
            csl = slice(s * U, (s + 1) * U)
            gsl = grid[b][:].rearrange("p (g c) -> p g c", c=COLS)[:, :, csl]
            v.tensor_tensor(out=gsl, in0=inter[b][:], in1=ssum[b][:],
                            op=Alu.subtract)   # t_log
            # colmax partial: elementwise max accumulate over supertiles
            i3 = grid[b][:].rearrange("p (g c) -> p g c", c=COLS)[:, :, csl]
            if s == 0:
                v.tensor_tensor(out=colaccW[b][:], in0=i3, in1=i3, op=Alu.max)
            else:
                caw = colaccW[b][:].rearrange("p (g c) -> p g c", c=U)
                v.tensor_tensor(out=caw, in0=caw, in1=i3, op=Alu.max)
            # rowmax over g via fold tree (fp16 2x)
            rf = rfold[b][:].rearrange("p (g c) -> p g c", c=U)
            v.tensor_tensor(out=rf[:, 0:16, :], in0=i3[:, 0:16, :],
                            in1=i3[:, 16:32, :], op=Alu.max)
            g2 = 16
            while g2 > 2:
                h = g2 // 2
                v.tensor_tensor(out=rf[:, 0:h, :], in0=rf[:, 0:h, :],
                                in1=rf[:, h:g2, :], op=Alu.max)
                g2 = h
            v.tensor_tensor(out=rowmax[b][:, csl], in0=rf[:, 0, :],
                            in1=rf[:, 1, :], op=Alu.max)

        def grid_stage_b(b, s):
            csl = slice(s * U, (s + 1) * U)
            t2 = grid[b][:].rearrange("p (g c) -> p g c", c=COLS)[:, :, csl]
            rmb = rowmax[b][:, csl].unsqueeze(1).to_broadcast([P, G, U])
            # write is-max in transpose-friendly layout: [cg][g][c4] so each
            # contiguous 128-block is one (g, c4) transpose chunk
            imt = ismax[b][:].rearrange("p (cg g r) -> p g cg r", g=G, r=4)
            v.tensor_tensor(out=imt, in0=t2, in1=rmb, op=Alu.is_equal)
            mout = pspool.tile([P, 512], F32, name=f"mout{b}", tag=f"mout{b}")
            for q in range(4):
                tq = tsb[b * 2 + q % 2]
                tp = pspool.tile([P, 512], F16, name=f"tp{b}_{q % 2}",
                                 tag=f"tp{b}_{q % 2}")
                for j in range(4):
                    cj = q * 4 + j
                    pe.transpose(tp[:, j * P:(j + 1) * P],
                                 ismax[b][:, cj * P:(cj + 1) * P],
                                 ident[:])
                sc.copy(tq[:], tp[:])
                for j in range(4):
                    cj = q * 4 + j
                    nc.tensor.matmul(mout[:, cj * G:(cj + 1) * G],
                                     tq[:, j * P:(j + 1) * P],
                                     gtmat[b][:])
            m_src = mout[:].rearrange("p (c v) -> p v c", v=8)
            m_dst = matched2[:].rearrange("p (v i c) -> p v i c", i=IMG,
                                          c=COLS)[:, :, b, csl]
            sc.copy(m_dst, m_src)

        for s in range(NSUP):
            grid_stage_a(0, s)
            grid_s_add(0, s)
            grid_lns(0, s)
            grid_stage_a(1, s)
            grid_s_add(1, s)
            grid_lns(1, s)
            if s > 0:
                grid_stage_a2(0, s - 1)
                grid_stage_b(0, s - 1)
                grid_stage_a2(1, s - 1)
                grid_stage_b(1, s - 1)
        for b in range(IMG):
            grid_stage_a2(b, NSUP - 1)
            grid_stage_b(b, NSUP - 1)

        # ---------------- colmax finalize ----------------
        for b in range(IMG):
            caw = colaccW[b][:].rearrange("p (g c) -> p g c", c=U)
            w = U
            while w > 1:
                h = w // 2
                v.tensor_tensor(out=caw[:, :, 0:h], in0=caw[:, :, 0:h],
                                in1=caw[:, :, h:w], op=Alu.max)
                w = h
            v.tensor_copy(colacc[b][:], caw[:, :, 0])
            ct_ps = pspool.tile([G, P], F16, name="ct_ps", tag="pss")
            pe.transpose(ct_ps[:], colacc[b][:], ident[:])
            v.tensor_copy(colT[b][:], ct_ps[:])
            v.tensor_reduce(out=cmax_col[b][:], in_=colT[b][:], axis=Ax.X,
                            op=Alu.max)
            cm_ps = pspool.tile([1, G], F16, name="cm_ps", tag="pss")
            pe.transpose(cm_ps[:], cmax_col[b][:], ident[:G, :G])
            v.tensor_copy(cm_row[b][:], cm_ps[:])
            pbcast(cmb[b][:], cm_row[b][:], n=G, f16=True)
            expand4(cmb4[b][:], cmb[b][:], G)

        # ---------------- forced pass: eq one-hot via the matched-gt PE path
        # eq = (t == cmb[g]); transpose + gtmat matmul; the count column
        # (ci*8+7) gives per-anchor hit counts. Reuses ismax tiles/psum tags.
        for s in range(NSUP):
            for b in range(IMG):
                csl = slice(s * U, (s + 1) * U)
                t2 = grid[b][:].rearrange("p (g c) -> p g c", c=COLS)[:, :, csl]
                cm = cmb4[b][:].rearrange("p (g r) -> p g r", r=4) \
                    .unsqueeze(2).to_broadcast([P, G, U // 4, 4])
                imt = ismax[b][:].rearrange("p (cg g r) -> p g cg r", g=G, r=4)
                v.tensor_tensor(out=imt, in0=t2, in1=cm, op=Alu.is_equal)
                mout = pspool.tile([P, 512], F32, name=f"mout{b}",
                                   tag=f"mout{b}")
                for q in range(4):
                    tq = tsb[b * 2 + q % 2]
                    tp = pspool.tile([P, 512], F16, name=f"tp{b}_{q % 2}",
                                     tag=f"tp{b}_{q % 2}")
                    for j in range(4):
                        cj = q * 4 + j
                        pe.transpose(tp[:, j * P:(j + 1) * P],
                                     ismax[b][:, cj * P:(cj + 1) * P],
                                     ident[:])
                    sc.copy(tq[:], tp[:])
                    for j in range(4):
                        cj = q * 4 + j
                        nc.tensor.matmul(mout[:, cj * G:(cj + 1) * G],
                                         tq[:, j * P:(j + 1) * P],
                                         gtmat[b][:])
                m_cnt = mout[:].rearrange("p (c v) -> p v c", v=8)[:, 7, :]
                sc.copy(forcedc[b][:, csl], m_cnt)

        # ---------------- matched normalize (fp16 tie fix) -----------------
        # matched planes: 0..3 coords, 4 area, 5 cx, 6 cy, 7 count
        m2v = matched2[:].rearrange("p (v ic) -> p v ic", v=8)
        sc.activation(s0[:], m2v[:, 7, :], Act.Ln)
        sc.activation(s0[:], s0[:], Act.Exp, scale=-1.0)
        rc = s0[:].unsqueeze(1).to_broadcast([P, 7, IMG * COLS])
        v.tensor_tensor(out=matched2[:, 0:7 * IMG * COLS],
                        in0=matched2[:, 0:7 * IMG * COLS], in1=rc, op=Alu.mult)

        # ---------------- DIoU loc loss (batched over img) ------------------
        w0r = w0[:].rearrange("p (k ic) -> p k ic", k=2)
        w03 = w0[:].rearrange("p (k i c) -> p k i c", i=IMG, c=COLS)
        w13 = w1_[:].rearrange("p (k i c) -> p k i c", i=IMG, c=COLS)
        bbk = bbox[:].rearrange("p (k ic) -> p k ic", k=4)
        s13 = s1[:].rearrange("p (i c) -> p i c", c=COLS)
        pos3 = pos2[:].rearrange("p (i c) -> p i c", c=COLS)

        v.tensor_tensor(out=w0[:], in0=bbox[:, 0:2 * IMG * COLS],
                        in1=matched2[:, 0:2 * IMG * COLS], op=Alu.max)
        v.tensor_tensor(out=w1_[:], in0=bbox[:, 2 * IMG * COLS:],
                        in1=matched2[:, 2 * IMG * COLS:4 * IMG * COLS],
                        op=Alu.min)
        v.tensor_tensor(out=w0[:], in0=w1_[:], in1=w0[:], op=Alu.subtract)
        sc.activation(w0[:], w0[:], Act.Relu)
        v.tensor_tensor(out=s0[:], in0=w0r[:, 0, :], in1=w0r[:, 1, :],
                        op=Alu.mult)                      # inter
        v.tensor_tensor(out=w0r[:, 0, :], in0=bbk[:, 2, :], in1=bbk[:, 0, :],
                        op=Alu.subtract)
        v.tensor_tensor(out=w0r[:, 1, :], in0=bbk[:, 3, :], in1=bbk[:, 1, :],
                        op=Alu.subtract)
        v.tensor_tensor(out=s1[:], in0=w0r[:, 0, :], in1=w0r[:, 1, :],
                        op=Alu.mult)                      # areaP
        v.tensor_tensor(out=s1[:], in0=s1[:], in1=m2v[:, 4, :], op=Alu.add)
        v.tensor_tensor(out=s1[:], in0=s1[:], in1=s0[:], op=Alu.subtract)
        sc.activation(s0[:], s0[:], Act.Ln, bias=lnb_c[:])
        sc.activation(s1[:], s1[:], Act.Ln)
        v.tensor_tensor(out=s0[:], in0=s0[:], in1=s1[:], op=Alu.subtract)
        sc.activation(s0[:], s0[:], Act.Exp)                   # iou
        v.tensor_tensor(out=w0[:], in0=bbox[:, 0:2 * IMG * COLS],
                        in1=matched2[:, 0:2 * IMG * COLS], op=Alu.min)
        v.tensor_tensor(out=w1_[:], in0=bbox[:, 2 * IMG * COLS:],
                        in1=matched2[:, 2 * IMG * COLS:4 * IMG * COLS],
                        op=Alu.max)
        v.tensor_tensor(out=w0[:], in0=w1_[:], in1=w0[:], op=Alu.subtract)
        sc.activation(w0[:], w0[:], Act.Square, scale=float(CSC))
        v.tensor_tensor(out=s2[:], in0=w0r[:, 0, :], in1=w0r[:, 1, :],
                        op=Alu.add)                       # c2 (scaled)
        v.tensor_tensor(out=w0r[:, 0, :], in0=bbk[:, 0, :], in1=bbk[:, 2, :],
                        op=Alu.add)
        v.tensor_tensor(out=w0r[:, 1, :], in0=bbk[:, 1, :], in1=bbk[:, 3, :],
                        op=Alu.add)
        v.tensor_tensor(out=w0[:], in0=w0[:],
                        in1=m2v[:, 5:7, :].rearrange("p v ic -> p (v ic)"),
                        op=Alu.subtract)
        sc.activation(w0[:], w0[:], Act.Square, scale=float(0.5 * CSC))
        sc.activation(s2[:], s2[:], Act.Ln)
        v.tensor_tensor(out=s1[:], in0=w0r[:, 0, :], in1=w0r[:, 1, :],
                        op=Alu.add)                       # d2 (scaled)
        sc.activation(s1[:], s1[:], Act.Ln, bias=lnb_c[:])
        v.tensor_tensor(out=s1[:], in0=s1[:], in1=s2[:], op=Alu.subtract)
        sc.activation(s1[:], s1[:], Act.Exp)              # d2/c2
        v.scalar_tensor_tensor(out=s1[:], in0=s0[:], scalar=-1.0, in1=s1[:],
                               op0=Alu.mult, op1=Alu.add)  # d2/c2 - iou
        v.tensor_scalar(s1[:], s1[:], 1.0, 100.0, Alu.add, Alu.min)  # loc

        # ---------------- pos / n_pos (forced now complete) -----------------
        for b in range(IMG):
            v.tensor_scalar(s4[b][:], rowmax[b][:], float(LN_THIRD), None,
                            Alu.is_gt)
            v.tensor_scalar(s5[b][:], forcedc[b][:], 0.5, None, Alu.is_ge)
            v.tensor_tensor(out=pos3[:, b, :], in0=s4[b][:], in1=s5[b][:],
                            op=Alu.max)
            v.tensor_reduce(out=npp[b][:], in_=pos3[:, b, :], axis=Ax.X,
                            op=Alu.add)
            v.scalar_tensor_tensor(out=s4[b][:], in0=s13[:, b, :], scalar=0.0,
                                   in1=pos3[:, b, :], op0=Alu.add, op1=Alu.mult,
                                   accum_out=locsum_pp[b][:])

        # ---------------- focal conf loss (batched) -------------------------
        sc.activation(s1[:], conf[:], Act.Exp)
        sc.activation(s1[:], s1[:], Act.Ln, bias=1.0)    # softplus
        v.tensor_tensor(out=s0[:], in0=conf[:], in1=s1[:], op=Alu.subtract)
        sc.activation(s0[:], s0[:], Act.Exp)             # sigmoid
        v.tensor_scalar(s3[:], pos2[:], -2.0, 1.0, Alu.mult, Alu.add)  # 1-2t
        v.tensor_tensor(out=s3[:], in0=s0[:], in1=s3[:], op=Alu.mult)
        v.tensor_tensor(out=s3[:], in0=s3[:], in1=pos2[:], op=Alu.add)  # 1-p_t
        v.tensor_tensor(out=s2[:], in0=conf[:], in1=pos2[:], op=Alu.mult)
        v.tensor_tensor(out=s2[:], in0=s1[:], in1=s2[:], op=Alu.subtract)  # ce
        v.tensor_tensor(out=s3[:], in0=s3[:], in1=s3[:], op=Alu.mult)
        v.tensor_tensor(out=s2[:], in0=s3[:], in1=s2[:], op=Alu.mult)
        v.tensor_scalar(s3[:], pos2[:], -0.5, 0.75, Alu.mult, Alu.add)
        v.tensor_tensor(out=s2[:], in0=s2[:], in1=s3[:], op=Alu.mult)
        v.tensor_scalar(s2[:], s2[:], 100.0, None, Alu.min)   # cl
        s23 = s2[:].rearrange("p (i c) -> p i c", c=COLS)
        nv3 = nv2[:].rearrange("p (i c) -> p i c", c=COLS)
        for b in range(IMG):
            v.scalar_tensor_tensor(out=s4[b][:], in0=s23[:, b, :], scalar=0.0,
                                   in1=pos3[:, b, :], op0=Alu.add, op1=Alu.mult,
                                   accum_out=possum_pp[b][:])
            v.tensor_tensor(out=nv3[:, b, :], in0=s23[:, b, :], in1=s4[b][:],
                            op=Alu.subtract)

        # ---------------- hard negative mining ------------------------------
        for b in range(IMG):
            v.tensor_reduce(out=maxv_pp[b][:], in_=nv3[:, b, :], axis=Ax.X,
                            op=Alu.max)
            mx_ps = pspool.tile([1, P], F32, name="mx_ps", tag="pss")
            pe.transpose(mx_ps[:], maxv_pp[b][:], ident32[:])
            v.tensor_copy(mx_row[b][:], mx_ps[:])
            v.tensor_reduce(out=maxv1[b][:], in_=mx_row[b][:], axis=Ax.X,
                            op=Alu.max)

            np_ps = pspool.tile([1, 1], F32, name="np_ps", tag="pss")
            nc.tensor.matmul(np_ps[:], ones_col[:], npp[b][:])
            v.tensor_copy(npos1[b][:], np_ps[:])
            v.tensor_scalar(k1[b][:], npos1[b][:], NEG_POS_RATIO, None, Alu.mult)
            v.tensor_scalar(k2[b][:], npos1[b][:], -1.0, float(A), Alu.mult,
                            Alu.add)
            v.tensor_tensor(out=kk[b][:], in0=k1[b][:], in1=k2[b][:], op=Alu.min)

            pbcast(maxvb[b][:], maxv1[b][:])
            v.tensor_scalar(w1c[b][:], maxvb[b][:], 1.0 / NBIN, None, Alu.mult)
        # img1 mining counts run on Act as sum(sign(nv - thr)) = 2*cnt - 512
        v.tensor_scalar(kk2[:], kk[1][:], 2.0, -float(A), Alu.mult, Alu.add)

        for lev in range(NLEV):
            for b in range(IMG):
                if lev == 0:
                    v.tensor_copy(wl[b][0][:], w1c[b][:])
                    v.tensor_scalar(thr[b][:], iota_f[:], wl[b][0][:], None,
                                    Alu.mult)
                else:
                    v.tensor_scalar(wl[b][lev][:], wl[b][lev - 1][:], 1.0 / NBIN,
                                    None, Alu.mult)
                    v.tensor_scalar(thr[b][:], iota_f[:], wl[b][lev][:],
                                    lo_b[b][lev - 1][:], Alu.mult, Alu.add)
                if b == 0:
                    for bn in range(NBIN):
                        v.tensor_scalar(sink[b][:], nv3[:, b, :],
                                        thr[b][:, bn:bn + 1], 0.0,
                                        Alu.is_gt, Alu.add,
                                        accum_out=cge[b][:, bn:bn + 1])
                else:
                    v.tensor_scalar(thrn[b][:], thr[b][:], -1.0, None, Alu.mult)
                    for bn in range(NBIN):
                        sc.activation(sink[b][:], nv3[:, b, :], Act.Sign,
                                      bias=thrn[b][:, bn:bn + 1],
                                      accum_out=cge[b][:, bn:bn + 1])
                cg_ps = pspool.tile([1, NBIN], F32, name="cg_ps", tag="pss")
                nc.tensor.matmul(cg_ps[:], ones_col[:], cge[b][:])
                v.tensor_copy(cget[b][:], cg_ps[:])
                kcmp = kk[b][:] if b == 0 else kk2[:]
                v.tensor_scalar(gek[b][:], cget[b][:], kcmp, 0.0, Alu.is_ge,
                                Alu.add, accum_out=scnt[b][:])
                v.tensor_scalar(lo_new[b][:], scnt[b][:], 1.0, wl[b][lev][0:1, :],
                                Alu.subtract, Alu.mult)
                v.tensor_scalar(tau[b][lev][:], scnt[b][:], wl[b][lev][0:1, :],
                                None, Alu.mult)
                if lev > 0:
                    v.tensor_tensor(out=lo_new[b][:], in0=lo_new[b][:],
                                    in1=lo_b[b][lev - 1][0:1, :], op=Alu.add)
                    v.tensor_tensor(out=tau[b][lev][:], in0=tau[b][lev][:],
                                    in1=lo_b[b][lev - 1][0:1, :], op=Alu.add)
                pbcast(lo_b[b][lev][:], lo_new[b][:])

        st_ps = [None, None]
        sm_ps = [None, None]
        for b in range(IMG):
            pbcast(tau_b[b][:], tau[b][NLEV - 1][:])
            v.tensor_scalar(s4[b][:], nv3[:, b, :], tau_b[b][:], 0.0, Alu.is_gt,
                            Alu.add, accum_out=cnt_pp[b][:])
            v.tensor_tensor(out=s5[b][:], in0=nv3[:, b, :], in1=s4[b][:],
                            op=Alu.mult)
            v.tensor_reduce(out=sum_pp[b][:], in_=s5[b][:], axis=Ax.X,
                            op=Alu.add)
            v.tensor_copy(stack[b][:, 0:1], npp[b][:])
            v.tensor_copy(stack[b][:, 1:2], locsum_pp[b][:])
            v.tensor_copy(stack[b][:, 2:3], possum_pp[b][:])
            v.tensor_copy(stack[b][:, 3:4], cnt_pp[b][:])
            st_ps[b] = pspool.tile([1, 4], F32, name=f"st_ps{b}", tag="pss")
            nc.tensor.matmul(st_ps[b][:], ones_col[:], stack[b][:])
            sm_ps[b] = pspool.tile([1, 1], F32, name=f"sm_ps{b}", tag="pss")
            nc.tensor.matmul(sm_ps[b][:], ones_col[:], sum_pp[b][:])
        for b in range(IMG):
            v.tensor_copy(res_sb[b][:, 0:4], st_ps[b][:])
            v.tensor_copy(res_sb[b][:, 4:5], sm_ps[b][:])
            v.tensor_copy(res_sb[b][:, 5:6], tau[b][NLEV - 1][:])
            v.tensor_copy(res_sb[b][:, 6:7], maxv1[b][:])
            v.tensor_copy(res_sb[b][:, 7:8], kk[b][:])
            nc.sync.dma_start(res_d[b], res_sb[b][:])

    nc.compile()
    return nc


_NC_CACHE = None


def _get_nc():
    global _NC_CACHE
    if _NC_CACHE is None:
        _NC_CACHE = _build_nc()
    return _NC_CACHE


def _make_in_maps(inputs):
    bbox_pred = np.asarray(inputs["bbox_pred"], dtype=np.float32)
    conf_pred = np.asarray(inputs["conf_pred"], dtype=np.float32)
    anchors = np.asarray(inputs["anchors"], dtype=np.float32)
    gt_boxes = np.asarray(inputs["gt_boxes"], dtype=np.float32)

    anch_h = np.ascontiguousarray(
        anchors.reshape(P, COLS, 4).transpose(2, 0, 1).astype(np.float16))
    in_maps = []
    for i in range(NCORE):
        bsl = slice(IMG * i, IMG * (i + 1))
        bb = bbox_pred[bsl].reshape(IMG, P, COLS, 4).transpose(0, 3, 1, 2)
        gt = gt_boxes[bsl]                       # [IMG, G, 4]
        gt_k = gt.transpose(0, 2, 1).reshape(IMG, 1, 4 * G)
        area = (gt[:, :, 2] - gt[:, :, 0]) * (gt[:, :, 3] - gt[:, :, 1])
        cx = gt[:, :, 0] + gt[:, :, 2]
        cy = gt[:, :, 1] + gt[:, :, 3]
        gt8 = np.concatenate([gt, area[..., None], cx[..., None], cy[..., None],
                              np.ones_like(area)[..., None]], axis=2)
        gtm = np.zeros((IMG, P, G), dtype=np.float32)
        for ci in range(4):
            gtm[:, ci::4, ci * 8:(ci + 1) * 8] = gt8
        in_maps.append({
            "anch": anch_h,
            "bbox": np.ascontiguousarray(bb.astype(np.float16)),
            "conf": np.ascontiguousarray(
                conf_pred[bsl].reshape(IMG, P, COLS).astype(np.float16)),
            "gtb": np.ascontiguousarray(gt_k.astype(np.float16)),
            "gtm": np.ascontiguousarray(gtm.astype(np.float16)),
        })
    return in_maps


def kernel(bbox_pred, conf_pred, anchors, gt_boxes):
    nc = _get_nc()
    in_maps = _make_in_maps(dict(bbox_pred=bbox_pred, conf_pred=conf_pred,
                                 anchors=anchors, gt_boxes=gt_boxes))
    out = run_bass_kernel_spmd(nc, in_maps, core_ids=list(range(NCORE)))

    loc_total = np.float32(0.0)
    conf_total = np.float32(0.0)
    npos_total = np.float32(0.0)
    for i in range(NCORE):
        res = out.results[i]["res"]  # [IMG, 1, 8]
        for b in range(IMG):
            npos, locsum, possum, cnt_gt, sum_gt, tau_hi, maxv, kdev = \
                [np.float32(x) for x in res[b, 0, :8]]
            k = np.float32(min(NEG_POS_RATIO * npos, A - npos))
            wl_last = np.float32(maxv / NBIN ** NLEV)
            rem = max(np.float32(0.0), np.float32(k - cnt_gt))
            neg = np.float32(sum_gt + rem * (tau_hi - wl_last * np.float32(0.5)))
            loc_total = np.float32(loc_total + locsum)
            conf_total = np.float32(conf_total + possum + neg)
            npos_total = np.float32(npos_total + npos)
    num_pos = np.float32(max(1.0, npos_total))
    loc_loss = np.float32(loc_total / num_pos)
    conf_loss = np.float32(conf_total / num_pos)
    return (np.float32(loc_loss + conf_loss), conf_loss, loc_loss)


# revision 30
# speedup vs baseline: 1.0024x; 1.0024x over previous
"""Trainium2 Bass kernel for nn_DetectionLoss (SSD-style detection loss).

Data-parallel over the batch: 16 images, 8 NeuronCores, 2 images per core.

v2 design (vs the f32 baseline):
- The [A, G] match grid is computed in fp16 in a g-major layout
  [partition, gt, anchor-col] so every heavy DVE op runs in the 2x fp16
  mode with a packed last AP dim. Broadcast operands that vary along gt
  use width-4 replicated tiles (gt_rep4 / areaG_rep4 / cmb_rep4) so the
  last AP dim stays packed.
- t = inter / (areaA + areaG + eps) replaces IoU: iou = t/(1-t) is a
  monotone map, so row/col argmax and the iou>0.5 threshold (t > 1/3)
  are preserved while the union subtraction disappears.
- Engine split: Act does relu + reciprocal + PSUM->SBUF copies, Pool
  (gpsimd) does the segmented reduces, PE does transposes + the
  matched-gt one-hot matmul (8 cols: x1,y1,x2,y2,area,cx,cy,count).
  Division by the count column fixes fp16 is-max ties (averages tied
  gts, which have equal IoU to within fp16 resolution).
- The two images' grid supertiles are software-pipelined (stage A:
  elementwise through t; stage B: is-max + transpose/matmul) so the DVE
  never waits on the Pool row-max.
- DIoU + focal run batched over both images in planar fp16 layout;
  enclosing-box / center squares are scaled by 1/16 to stay in range.
- Hard-negative mining: 2-level x 16-bin threshold search (resolution
  256) with fp16 counting split across DVE and Pool; host applies the
  exact-sum + midpoint boundary-bin correction.
"""
import sys

sys.path.insert(0, '/opt/trn_rl_repo')

import numpy as np
import concourse.bass as bass
import concourse.bacc as bacc
import concourse.mybir as mybir
from concourse.tile import TileContext
from concourse.bass_utils import run_bass_kernel_spmd
from concourse.masks import make_identity
from contextlib import ExitStack

Alu = mybir.AluOpType
Act = mybir.ActivationFunctionType
Ax = mybir.AxisListType
F32 = mybir.dt.float32
F16 = mybir.dt.float16
I32 = mybir.dt.int32

P = 128
A = 65536
G = 32
IMG = 2            # images per core
NCORE = 8
COLS = A // P      # 512 anchor columns per partition
U = 64             # anchor columns per supertile
WG = U * G         # 2048 grid elems per supertile per partition
NSUP = COLS // U   # 8 supertiles
EPS = 1e-7
NBIN = 16          # histogram bins per mining level
NLEV = 2           # mining levels (resolution NBIN**NLEV = 256)
NEG_POS_RATIO = 3.0
LN_THIRD = -1.0986123  # iou > 0.5  <=>  ln(inter) - ln(S) > ln(1/3)
LNB = 6e-5             # ln bias: ln(x + LNB) keeps ln finite at x=0
CSC = 1.0 / 16.0   # coordinate scale before squaring in fp16 diou


def _build_nc():
    nc = bacc.Bacc("TRN2", target_bir_lowering=False, debug=False)
    anch_d = nc.dram_tensor("anch", [4, P, COLS], F16, kind="ExternalInput")
    bbox_d = nc.dram_tensor("bbox", [IMG, 4, P, COLS], F16, kind="ExternalInput")
    conf_d = nc.dram_tensor("conf", [IMG, P, COLS], F16, kind="ExternalInput")
    # gt coords k-major: [IMG, 1, 4*G]  ([k*G+g] = gt[g,k])
    gt_d = nc.dram_tensor("gtb", [IMG, 1, 4 * G], F16, kind="ExternalInput")
    # matched-gt matmul weights: rows r = g*4+ci, cols ci*8+v,
    # v in (x1,y1,x2,y2,area,cx,cy,1)
    gtm_d = nc.dram_tensor("gtm", [IMG, P, G], F16, kind="ExternalInput")
    res_d = nc.dram_tensor("res", [IMG, 1, 8], F32, kind="ExternalOutput")

    v = nc.vector
    sc = nc.scalar
    pe = nc.tensor
    gp = nc.gpsimd

    with TileContext(nc) as tc, ExitStack() as ctx:
        pool = ctx.enter_context(tc.tile_pool(name="main", bufs=1))
        pspool = ctx.enter_context(tc.tile_pool(name="ps", bufs=1, space="PSUM"))

        def T(name, cols, parts=P, dt=F16):
            return pool.tile([parts, cols], dt, name=name)

        def TF(name, cols, parts=P):
            return pool.tile([parts, cols], F32, name=name)

        # ---------------- persistent tiles ----------------
        anch = T("anch", 4 * COLS)              # planar [k][c]
        areaA = T("areaA", COLS)
        bbox = T("bbox", 4 * IMG * COLS)        # [k][img][c] planar
        conf = T("conf", IMG * COLS)            # [img][c]
        grid = [T(f"grid{b}", G * COLS) for b in range(IMG)]
        rowmax = [T(f"rowmax{b}", COLS) for b in range(IMG)]
        matched2 = T("matched2", 8 * IMG * COLS)    # planes [v][i][c]
        colacc = [T(f"colacc{b}", G) for b in range(IMG)]
        colaccW = [T(f"colaccW{b}", G * U) for b in range(IMG)]
        forcedc = [T(f"forcedc{b}", COLS) for b in range(IMG)]
        pos2 = T("pos2", IMG * COLS)            # [img][c]
        nv2 = T("nv2", IMG * COLS)              # [img][c]
        gtall = [T(f"gtall{b}", 4 * G) for b in range(IMG)]   # [k][g]
        gtmat = [T(f"gtmat{b}", G) for b in range(IMG)]       # [128, 32]
        gt_rep4 = [T(f"gt_rep4{b}", 4 * G * 4) for b in range(IMG)]  # [kg][4]
        areaG4 = [T(f"areaG4{b}", G * 4) for b in range(IMG)]        # [g][4]
        cmb4 = [T(f"cmb4{b}", G * 4) for b in range(IMG)]            # [g][4]
        sG = [T(f"sG{b}", G) for b in range(IMG)]

        # stage-A scratch (per image)
        lt = [T(f"lt{i}", 2 * WG) for i in range(IMG)]
        rb = [T(f"rb{i}", 2 * WG) for i in range(IMG)]
        inter = [T(f"inter{i}", WG) for i in range(IMG)]
        ssum = [T(f"ssum{i}", WG) for i in range(IMG)]
        # stage-B scratch (per image)
        ismax = [T(f"ismax{i}", WG) for i in range(IMG)]
        tsb = [T(f"tsb{i}", 4 * P) for i in range(4)]   # [img*2 + q%2]
        rfold = [T(f"rfold{i}", G * U // 2) for i in range(IMG)]

        # diou/focal scratch (batched over img)
        w0 = T("w0", 2 * IMG * COLS)
        w1_ = T("w1_", 2 * IMG * COLS)
        s0 = T("s0", IMG * COLS)
        s1 = T("s1", IMG * COLS)
        s2 = T("s2", IMG * COLS)
        s3 = T("s3", IMG * COLS)
        s4 = [T(f"s4_{b}", COLS) for b in range(IMG)]
        s5 = [T(f"s5_{b}", COLS) for b in range(IMG)]

        ident = T("ident", P)                   # fp16 identity
        ident32 = TF("ident32", P)
        ones_col = TF("ones_col", 1)
        ones_row = TF("ones_row", P, parts=1)
        ones_row16 = T("ones_row16", P, parts=1)
        colT = [T(f"colT{b}", P, parts=G) for b in range(IMG)]
        cmax_col = [T(f"cmax_col{b}", 1, parts=G) for b in range(IMG)]
        cm_row = [T(f"cm_row{b}", G, parts=1) for b in range(IMG)]
        cmb = [T(f"cmb{b}", G) for b in range(IMG)]
        mx_row = [TF(f"mx_row{b}", P, parts=1) for b in range(IMG)]
        npp = [TF(f"npp{b}", 1) for b in range(IMG)]
        locsum_pp = [TF(f"locsum_pp{b}", 1) for b in range(IMG)]
        possum_pp = [TF(f"possum_pp{b}", 1) for b in range(IMG)]
        cnt_pp = [TF(f"cnt_pp{b}", 1) for b in range(IMG)]
        sum_pp = [TF(f"sum_pp{b}", 1) for b in range(IMG)]
        maxv_pp = [TF(f"maxv_pp{b}", 1) for b in range(IMG)]
        maxvb = [TF(f"maxvb{b}", 1) for b in range(IMG)]
        w1c = [TF(f"w1c{b}", 1) for b in range(IMG)]
        tau_b = [TF(f"tau_b{b}", 1) for b in range(IMG)]
        stack = [TF(f"stack{b}", 4) for b in range(IMG)]
        iota_f = TF("iota_f", NBIN)
        thr = [TF(f"thr{b}", NBIN) for b in range(IMG)]
        cge = [TF(f"cge{b}", NBIN) for b in range(IMG)]
        sink = [T(f"sink{b}", COLS) for b in range(IMG)]
        wl = [[TF(f"wl{b}_{l}", 1) for l in range(NLEV)] for b in range(IMG)]
        lo_b = [[TF(f"lo_b{b}_{l}", 1) for l in range(NLEV)] for b in range(IMG)]
        cget = [TF(f"cget{b}", NBIN, parts=1) for b in range(IMG)]
        gek = [TF(f"gek{b}", NBIN, parts=1) for b in range(IMG)]
        scnt = [TF(f"scnt{b}", 1, parts=1) for b in range(IMG)]
        lo_new = [TF(f"lo_new{b}", 1, parts=1) for b in range(IMG)]
        tau = [[TF(f"tau{b}_{l}", 1, parts=1) for l in range(NLEV)]
               for b in range(IMG)]
        maxv1 = [TF(f"maxv1{b}", 1, parts=1) for b in range(IMG)]
        npos1 = [TF(f"npos1{b}", 1, parts=1) for b in range(IMG)]
        k1 = [TF(f"k1{b}", 1, parts=1) for b in range(IMG)]
        k2 = [TF(f"k2{b}", 1, parts=1) for b in range(IMG)]
        kk = [TF(f"kk{b}", 1, parts=1) for b in range(IMG)]
        kk2 = TF("kk2", 1, parts=1)
        thrn = [TF(f"thrn{b}", NBIN) for b in range(IMG)]
        res_sb = [TF(f"res_sb{b}", 8, parts=1) for b in range(IMG)]
        iota_i = pool.tile([P, NBIN], I32, name="iota_i")

        # ---------------- constants & loads ----------------
        anchsb = anch[:].rearrange("p (k c) -> p k c", c=COLS)
        for k in range(4):
            nc.sync.dma_start(anchsb[:, k, :], anch_d[k])
        bbsb = bbox[:].rearrange("p (k i c) -> p k i c", i=IMG, c=COLS)
        cfsb = conf[:].rearrange("p (i c) -> p i c", c=COLS)
        for b in range(IMG):
            for k in range(4):
                nc.sync.dma_start(bbsb[:, k, b, :], bbox_d[b, k])
            nc.sync.dma_start(cfsb[:, b, :], conf_d[b])
            nc.sync.dma_start(gtall[b][:],
                              gt_d[b].squeeze(0).partition_broadcast(P))
            nc.sync.dma_start(gtmat[b][:], gtm_d[b])
        lnb_c = TF("lnb_c", 1)
        v.memset(lnb_c[:], float(LNB))
        v.memset(ones_col[:], 1.0)
        v.memset(ones_row[:], 1.0)
        v.memset(ones_row16[:], 1.0)
        make_identity(nc, ident[:])
        make_identity(nc, ident32[:])
        nc.gpsimd.iota(iota_i[:], pattern=[[1, NBIN]], base=0, channel_multiplier=0)
        v.tensor_copy(iota_f[:], iota_i[:])

        anch3 = anch[:].rearrange("p (k c) -> p k c", c=COLS)
        v.tensor_tensor(out=s4[0][:], in0=anch3[:, 2, :], in1=anch3[:, 0, :],
                        op=Alu.subtract)
        v.tensor_tensor(out=s5[0][:], in0=anch3[:, 3, :], in1=anch3[:, 1, :],
                        op=Alu.subtract)
        v.tensor_tensor(out=areaA[:], in0=s4[0][:], in1=s5[0][:], op=Alu.mult)

        def pbcast(dst, src_row, n=1, f16=False):
            """Broadcast a [1, n] partition-0 row to [P, n] via a K=1 matmul."""
            bc_ps = pspool.tile([P, G], F32, name="bc_ps", tag="pss")
            orow = ones_row16 if f16 else ones_row
            nc.tensor.matmul(bc_ps[:, 0:n], orow[:], src_row)
            v.tensor_copy(dst, bc_ps[:, 0:n])

        def expand4(dst, src, n):
            """dst[p, n*4] <- src[p, n] replicated 4x along a new inner dim."""
            d3 = dst.rearrange("p (n r) -> p n r", r=4)
            v.tensor_copy(d3, src.unsqueeze(2).to_broadcast([P, n, 4]))

        # ---------------- per-image gt setup ----------------
        for b in range(IMG):
            ga = gtall[b][:].rearrange("p (k g) -> p k g", g=G)
            v.tensor_tensor(out=s4[b][:, 0:G], in0=ga[:, 2, :], in1=ga[:, 0, :],
                            op=Alu.subtract)
            v.tensor_tensor(out=s5[b][:, 0:G], in0=ga[:, 3, :], in1=ga[:, 1, :],
                            op=Alu.subtract)
            v.tensor_tensor(out=sG[b][:], in0=s4[b][:, 0:G], in1=s5[b][:, 0:G],
                            op=Alu.mult)
            expand4(gt_rep4[b][:], gtall[b][:], 4 * G)
            expand4(areaG4[b][:], sG[b][:], G)

        # ---------------- grid phase (pipelined over both images) ----------
        def grid_stage_a(b, s):
            csl = slice(s * U, (s + 1) * U)
            lt4 = lt[b][:].rearrange("p (kg c) -> p kg c", c=U)
            rb4 = rb[b][:].rearrange("p (kg c) -> p kg c", c=U)
            a_lo = anch3[:, 0:2, csl].unsqueeze(2).to_broadcast([P, 2, G, U])
            a_hi = anch3[:, 2:4, csl].unsqueeze(2).to_broadcast([P, 2, G, U])
            gr = gt_rep4[b][:].rearrange("p (kg r) -> p kg r", r=4)
            g_lo = gr[:, 0:2 * G, :].unsqueeze(2) \
                .to_broadcast([P, 2 * G, U // 4, 4])
            g_hi = gr[:, 2 * G:4 * G, :].unsqueeze(2) \
                .to_broadcast([P, 2 * G, U // 4, 4])
            v.tensor_tensor(out=lt4, in0=a_lo, in1=g_lo, op=Alu.max)
            v.tensor_tensor(out=rb4, in0=a_hi, in1=g_hi, op=Alu.min)
            v.tensor_tensor(out=lt[b][:], in0=rb[b][:], in1=lt[b][:],
                            op=Alu.subtract)
            sc.activation(rb[b][:], lt[b][:], Act.Relu)   # wh
            wh4 = rb[b][:].rearrange("p (k gc) -> p k gc", k=2)
            gp.tensor_tensor(out=inter[b][:], in0=wh4[:, 0, :], in1=wh4[:, 1, :],
                             op=Alu.mult)

        def grid_s_add(b, s):
            csl = slice(s * U, (s + 1) * U)
            aA = areaA[:, csl].unsqueeze(1).to_broadcast([P, G, U])
            aG = areaG4[b][:].rearrange("p (g r) -> p g r", r=4) \
                .unsqueeze(2).to_broadcast([P, G, U // 4, 4])
            seng = v if b == 0 else gp
            seng.tensor_tensor(out=ssum[b][:], in0=aA, in1=aG, op=Alu.add)

        def grid_lns(b, s):
            sc.activation(inter[b][:], inter[b][:], Act.Ln, bias=lnb_c[:])
            sc.activation(ssum[b][:], ssum[b][:], Act.Ln)

        def grid_stage_a2(b, s):
            csl = slice(s * U, (s + 1) * U)
            gsl = grid[b][:].rearrange("p (g c) -> p g c", c=COLS)[:, :, csl]
            v.tensor_tensor(out=gsl, in0=inter[b][:], in1=ssum[b][:],
                            op=Alu.subtract)   # t_log
            # colmax partial: elementwise max accumulate over supertiles
            i3 = grid[b][:].rearrange("p (g c) -> p g c", c=COLS)[:, :, csl]
            if s == 0:
                v.tensor_tensor(out=colaccW[b][:], in0=i3, in1=i3, op=Alu.max)
            else:
                caw = colaccW[b][:].rearrange("p (g c) -> p g c", c=U)
                v.tensor_tensor(out=caw, in0=caw, in1=i3, op=Alu.max)
            # rowmax over g via fold tree (fp16 2x)
            rf = rfold[b][:].rearrange("p (g c) -> p g c", c=U)
            v.tensor_tensor(out=rf[:, 0:16, :], in0=i3[:, 0:16, :],
                            in1=i3[:, 16:32, :], op=Alu.max)
            g2 = 16
            while g2 > 2:
                h = g2 // 2
                v.tensor_tensor(out=rf[:, 0:h, :], in0=rf[:, 0:h, :],
                                in1=rf[:, h:g2, :], op=Alu.max)
                g2 = h
            v.tensor_tensor(out=rowmax[b][:, csl], in0=rf[:, 0, :],
                            in1=rf[:, 1, :], op=Alu.max)

        def grid_stage_b(b, s):
            csl = slice(s * U, (s + 1) * U)
            t2 = grid[b][:].rearrange("p (g c) -> p g c", c=COLS)[:, :, csl]
            rmb = rowmax[b][:, csl].unsqueeze(1).to_broadcast([P, G, U])
            # write is-max in transpose-friendly layout: [cg][g][c4] so each
            # contiguous 128-block is one (g, c4) transpose chunk
            imt = ismax[b][:].rearrange("p (cg g r) -> p g cg r", g=G, r=4)
            v.tensor_tensor(out=imt, in0=t2, in1=rmb, op=Alu.is_equal)
            mout = pspool.tile([P, 512], F32, name=f"mout{b}", tag=f"mout{b}")
            for q in range(4):
                tq = tsb[b * 2 + q % 2]
                tp = pspool.tile([P, 512], F16, name=f"tp{b}_{q % 2}",
                                 tag=f"tp{b}_{q % 2}")
                for j in range(4):
                    cj = q * 4 + j
                    pe.transpose(tp[:, j * P:(j + 1) * P],
                                 ismax[b][:, cj * P:(cj + 1) * P],
                                 ident[:])
                sc.copy(tq[:], tp[:])
                for j in range(4):
                    cj = q * 4 + j
                    nc.tensor.matmul(mout[:, cj * G:(cj + 1) * G],
                                     tq[:, j * P:(j + 1) * P],
                                     gtmat[b][:])
            m_src = mout[:].rearrange("p (c v) -> p v c", v=8)
            m_dst = matched2[:].rearrange("p (v i c) -> p v i c", i=IMG,
                                          c=COLS)[:, :, b, csl]
            sc.copy(m_dst, m_src)

        for s in range(NSUP):
            grid_s_add(1, s)
            grid_stage_a(0, s)
            grid_s_add(0, s)
            grid_stage_a(1, s)
            grid_lns(0, s)
            grid_lns(1, s)
            if s > 0:
                grid_stage_a2(0, s - 1)
                grid_stage_b(0, s - 1)
                grid_stage_a2(1, s - 1)
                grid_stage_b(1, s - 1)
        for b in range(IMG):
            grid_stage_a2(b, NSUP - 1)
            grid_stage_b(b, NSUP - 1)

        # ---------------- colmax finalize ----------------
        for b in range(IMG):
            caw = colaccW[b][:].rearrange("p (g c) -> p g c", c=U)
            w = U
            while w > 1:
                h = w // 2
                v.tensor_tensor(out=caw[:, :, 0:h], in0=caw[:, :, 0:h],
                                in1=caw[:, :, h:w], op=Alu.max)
                w = h
            v.tensor_copy(colacc[b][:], caw[:, :, 0])
            ct_ps = pspool.tile([G, P], F16, name="ct_ps", tag="pss")
            pe.transpose(ct_ps[:], colacc[b][:], ident[:])
            v.tensor_copy(colT[b][:], ct_ps[:])
            v.tensor_reduce(out=cmax_col[b][:], in_=colT[b][:], axis=Ax.X,
                            op=Alu.max)
            cm_ps = pspool.tile([1, G], F16, name="cm_ps", tag="pss")
            pe.transpose(cm_ps[:], cmax_col[b][:], ident[:G, :G])
            v.tensor_copy(cm_row[b][:], cm_ps[:])
            pbcast(cmb[b][:], cm_row[b][:], n=G, f16=True)
            expand4(cmb4[b][:], cmb[b][:], G)

        # ---------------- forced pass: eq one-hot via the matched-gt PE path
        # eq = (t == cmb[g]); transpose + gtmat matmul; the count column
        # (ci*8+7) gives per-anchor hit counts. Reuses ismax tiles/psum tags.
        for s in range(NSUP):
            for b in range(IMG):
                csl = slice(s * U, (s + 1) * U)
                t2 = grid[b][:].rearrange("p (g c) -> p g c", c=COLS)[:, :, csl]
                cm = cmb4[b][:].rearrange("p (g r) -> p g r", r=4) \
                    .unsqueeze(2).to_broadcast([P, G, U // 4, 4])
                imt = ismax[b][:].rearrange("p (cg g r) -> p g cg r", g=G, r=4)
                v.tensor_tensor(out=imt, in0=t2, in1=cm, op=Alu.is_equal)
                mout = pspool.tile([P, 512], F32, name=f"mout{b}",
                                   tag=f"mout{b}")
                for q in range(4):
                    tq = tsb[b * 2 + q % 2]
                    tp = pspool.tile([P, 512], F16, name=f"tp{b}_{q % 2}",
                                     tag=f"tp{b}_{q % 2}")
                    for j in range(4):
                        cj = q * 4 + j
                        pe.transpose(tp[:, j * P:(j + 1) * P],
                                     ismax[b][:, cj * P:(cj + 1) * P],
                                     ident[:])
                    sc.copy(tq[:], tp[:])
                    for j in range(4):
                        cj = q * 4 + j
                        nc.tensor.matmul(mout[:, cj * G:(cj + 1) * G],
                                         tq[:, j * P:(j + 1) * P],
                                         gtmat[b][:])
                m_cnt = mout[:].rearrange("p (c v) -> p v c", v=8)[:, 7, :]
                sc.copy(forcedc[b][:, csl], m_cnt)

        # ---------------- matched normalize (fp16 tie fix) -----------------
        # matched planes: 0..3 coords, 4 area, 5 cx, 6 cy, 7 count
        m2v = matched2[:].rearrange("p (v ic) -> p v ic", v=8)
        sc.activation(s0[:], m2v[:, 7, :], Act.Ln)
        sc.activation(s0[:], s0[:], Act.Exp, scale=-1.0)
        rc = s0[:].unsqueeze(1).to_broadcast([P, 7, IMG * COLS])
        v.tensor_tensor(out=matched2[:, 0:7 * IMG * COLS],
                        in0=matched2[:, 0:7 * IMG * COLS], in1=rc, op=Alu.mult)

        # ---------------- DIoU loc loss (batched over img) ------------------
        w0r = w0[:].rearrange("p (k ic) -> p k ic", k=2)
        w03 = w0[:].rearrange("p (k i c) -> p k i c", i=IMG, c=COLS)
        w13 = w1_[:].rearrange("p (k i c) -> p k i c", i=IMG, c=COLS)
        bbk = bbox[:].rearrange("p (k ic) -> p k ic", k=4)
        s13 = s1[:].rearrange("p (i c) -> p i c", c=COLS)
        pos3 = pos2[:].rearrange("p (i c) -> p i c", c=COLS)

        v.tensor_tensor(out=w0[:], in0=bbox[:, 0:2 * IMG * COLS],
                        in1=matched2[:, 0:2 * IMG * COLS], op=Alu.max)
        v.tensor_tensor(out=w1_[:], in0=bbox[:, 2 * IMG * COLS:],
                        in1=matched2[:, 2 * IMG * COLS:4 * IMG * COLS],
                        op=Alu.min)
        v.tensor_tensor(out=w0[:], in0=w1_[:], in1=w0[:], op=Alu.subtract)
        sc.activation(w0[:], w0[:], Act.Relu)
        v.tensor_tensor(out=s0[:], in0=w0r[:, 0, :], in1=w0r[:, 1, :],
                        op=Alu.mult)                      # inter
        v.tensor_tensor(out=w0r[:, 0, :], in0=bbk[:, 2, :], in1=bbk[:, 0, :],
                        op=Alu.subtract)
        v.tensor_tensor(out=w0r[:, 1, :], in0=bbk[:, 3, :], in1=bbk[:, 1, :],
                        op=Alu.subtract)
        v.tensor_tensor(out=s1[:], in0=w0r[:, 0, :], in1=w0r[:, 1, :],
                        op=Alu.mult)                      # areaP
        v.tensor_tensor(out=s1[:], in0=s1[:], in1=m2v[:, 4, :], op=Alu.add)
        v.tensor_tensor(out=s1[:], in0=s1[:], in1=s0[:], op=Alu.subtract)
        sc.activation(s0[:], s0[:], Act.Ln, bias=lnb_c[:])
        sc.activation(s1[:], s1[:], Act.Ln)
        v.tensor_tensor(out=s0[:], in0=s0[:], in1=s1[:], op=Alu.subtract)
        sc.activation(s0[:], s0[:], Act.Exp)                   # iou
        v.tensor_tensor(out=w0[:], in0=bbox[:, 0:2 * IMG * COLS],
                        in1=matched2[:, 0:2 * IMG * COLS], op=Alu.min)
        v.tensor_tensor(out=w1_[:], in0=bbox[:, 2 * IMG * COLS:],
                        in1=matched2[:, 2 * IMG * COLS:4 * IMG * COLS],
                        op=Alu.max)
        v.tensor_tensor(out=w0[:], in0=w1_[:], in1=w0[:], op=Alu.subtract)
        sc.activation(w0[:], w0[:], Act.Square, scale=float(CSC))
        v.tensor_tensor(out=s2[:], in0=w0r[:, 0, :], in1=w0r[:, 1, :],
                        op=Alu.add)                       # c2 (scaled)
        v.tensor_tensor(out=w0r[:, 0, :], in0=bbk[:, 0, :], in1=bbk[:, 2, :],
                        op=Alu.add)
        v.tensor_tensor(out=w0r[:, 1, :], in0=bbk[:, 1, :], in1=bbk[:, 3, :],
                        op=Alu.add)
        v.tensor_tensor(out=w0[:], in0=w0[:],
                        in1=m2v[:, 5:7, :].rearrange("p v ic -> p (v ic)"),
                        op=Alu.subtract)
        sc.activation(w0[:], w0[:], Act.Square, scale=float(0.5 * CSC))
        sc.activation(s2[:], s2[:], Act.Ln)
        v.tensor_tensor(out=s1[:], in0=w0r[:, 0, :], in1=w0r[:, 1, :],
                        op=Alu.add)                       # d2 (scaled)
        sc.activation(s1[:], s1[:], Act.Ln, bias=lnb_c[:])
        v.tensor_tensor(out=s1[:], in0=s1[:], in1=s2[:], op=Alu.subtract)
        sc.activation(s1[:], s1[:], Act.Exp)              # d2/c2
        v.scalar_tensor_tensor(out=s1[:], in0=s0[:], scalar=-1.0, in1=s1[:],
                               op0=Alu.mult, op1=Alu.add)  # d2/c2 - iou
        v.tensor_scalar(s1[:], s1[:], 1.0, 100.0, Alu.add, Alu.min)  # loc

        # ---------------- pos / n_pos (forced now complete) -----------------
        for b in range(IMG):
            v.tensor_scalar(s4[b][:], rowmax[b][:], float(LN_THIRD), None,
                            Alu.is_gt)
            v.tensor_scalar(s5[b][:], forcedc[b][:], 0.5, None, Alu.is_ge)
            v.tensor_tensor(out=pos3[:, b, :], in0=s4[b][:], in1=s5[b][:],
                            op=Alu.max)
            v.tensor_reduce(out=npp[b][:], in_=pos3[:, b, :], axis=Ax.X,
                            op=Alu.add)
            v.scalar_tensor_tensor(out=s4[b][:], in0=s13[:, b, :], scalar=0.0,
                                   in1=pos3[:, b, :], op0=Alu.add, op1=Alu.mult,
                                   accum_out=locsum_pp[b][:])

        # ---------------- focal conf loss (batched) -------------------------
        sc.activation(s1[:], conf[:], Act.Exp)
        sc.activation(s1[:], s1[:], Act.Ln, bias=1.0)    # softplus
        v.tensor_tensor(out=s0[:], in0=conf[:], in1=s1[:], op=Alu.subtract)
        sc.activation(s0[:], s0[:], Act.Exp)             # sigmoid
        v.tensor_scalar(s3[:], pos2[:], -2.0, 1.0, Alu.mult, Alu.add)  # 1-2t
        v.tensor_tensor(out=s3[:], in0=s0[:], in1=s3[:], op=Alu.mult)
        v.tensor_tensor(out=s3[:], in0=s3[:], in1=pos2[:], op=Alu.add)  # 1-p_t
        v.tensor_tensor(out=s2[:], in0=conf[:], in1=pos2[:], op=Alu.mult)
        v.tensor_tensor(out=s2[:], in0=s1[:], in1=s2[:], op=Alu.subtract)  # ce
        v.tensor_tensor(out=s3[:], in0=s3[:], in1=s3[:], op=Alu.mult)
        v.tensor_tensor(out=s2[:], in0=s3[:], in1=s2[:], op=Alu.mult)
        v.tensor_scalar(s3[:], pos2[:], -0.5, 0.75, Alu.mult, Alu.add)
        v.tensor_tensor(out=s2[:], in0=s2[:], in1=s3[:], op=Alu.mult)
        v.tensor_scalar(s2[:], s2[:], 100.0, None, Alu.min)   # cl
        s23 = s2[:].rearrange("p (i c) -> p i c", c=COLS)
        nv3 = nv2[:].rearrange("p (i c) -> p i c", c=COLS)
        for b in range(IMG):
            v.scalar_tensor_tensor(out=s4[b][:], in0=s23[:, b, :], scalar=0.0,
                                   in1=pos3[:, b, :], op0=Alu.add, op1=Alu.mult,
                                   accum_out=possum_pp[b][:])
            v.tensor_tensor(out=nv3[:, b, :], in0=s23[:, b, :], in1=s4[b][:],
                            op=Alu.subtract)

        # ---------------- hard negative mining ------------------------------
        for b in range(IMG):
            v.tensor_reduce(out=maxv_pp[b][:], in_=nv3[:, b, :], axis=Ax.X,
                            op=Alu.max)
            mx_ps = pspool.tile([1, P], F32, name="mx_ps", tag="pss")
            pe.transpose(mx_ps[:], maxv_pp[b][:], ident32[:])
            v.tensor_copy(mx_row[b][:], mx_ps[:])
            v.tensor_reduce(out=maxv1[b][:], in_=mx_row[b][:], axis=Ax.X,
                            op=Alu.max)

            np_ps = pspool.tile([1, 1], F32, name="np_ps", tag="pss")
            nc.tensor.matmul(np_ps[:], ones_col[:], npp[b][:])
            v.tensor_copy(npos1[b][:], np_ps[:])
            v.tensor_scalar(k1[b][:], npos1[b][:], NEG_POS_RATIO, None, Alu.mult)
            v.tensor_scalar(k2[b][:], npos1[b][:], -1.0, float(A), Alu.mult,
                            Alu.add)
            v.tensor_tensor(out=kk[b][:], in0=k1[b][:], in1=k2[b][:], op=Alu.min)

            pbcast(maxvb[b][:], maxv1[b][:])
            v.tensor_scalar(w1c[b][:], maxvb[b][:], 1.0 / NBIN, None, Alu.mult)
        # img1 mining counts run on Act as sum(sign(nv - thr)) = 2*cnt - 512
        v.tensor_scalar(kk2[:], kk[1][:], 2.0, -float(A), Alu.mult, Alu.add)

        for lev in range(NLEV):
            for b in range(IMG):
                if lev == 0:
                    v.tensor_copy(wl[b][0][:], w1c[b][:])
                    v.tensor_scalar(thr[b][:], iota_f[:], wl[b][0][:], None,
                                    Alu.mult)
                else:
                    v.tensor_scalar(wl[b][lev][:], wl[b][lev - 1][:], 1.0 / NBIN,
                                    None, Alu.mult)
                    v.tensor_scalar(thr[b][:], iota_f[:], wl[b][lev][:],
                                    lo_b[b][lev - 1][:], Alu.mult, Alu.add)
                if b == 0:
                    for bn in range(NBIN):
                        v.tensor_scalar(sink[b][:], nv3[:, b, :],
                                        thr[b][:, bn:bn + 1], 0.0,
                                        Alu.is_gt, Alu.add,
                                        accum_out=cge[b][:, bn:bn + 1])
                else:
                    v.tensor_scalar(thrn[b][:], thr[b][:], -1.0, None, Alu.mult)
                    for bn in range(NBIN):
                        sc.activation(sink[b][:], nv3[:, b, :], Act.Sign,
                                      bias=thrn[b][:, bn:bn + 1],
                                      accum_out=cge[b][:, bn:bn + 1])
                cg_ps = pspool.tile([1, NBIN], F32, name="cg_ps", tag="pss")
                nc.tensor.matmul(cg_ps[:], ones_col[:], cge[b][:])
                v.tensor_copy(cget[b][:], cg_ps[:])
                kcmp = kk[b][:] if b == 0 else kk2[:]
                v.tensor_scalar(gek[b][:], cget[b][:], kcmp, 0.0, Alu.is_ge,
                                Alu.add, accum_out=scnt[b][:])
                v.tensor_scalar(lo_new[b][:], scnt[b][:], 1.0, wl[b][lev][0:1, :],
                                Alu.subtract, Alu.mult)
                v.tensor_scalar(tau[b][lev][:], scnt[b][:], wl[b][lev][0:1, :],
                                None, Alu.mult)
                if lev > 0:
                    v.tensor_tensor(out=lo_new[b][:], in0=lo_new[b][:],
                                    in1=lo_b[b][lev - 1][0:1, :], op=Alu.add)
                    v.tensor_tensor(out=tau[b][lev][:], in0=tau[b][lev][:],
                                    in1=lo_b[b][lev - 1][0:1, :], op=Alu.add)
                pbcast(lo_b[b][lev][:], lo_new[b][:])

        st_ps = [None, None]
        sm_ps = [None, None]
        for b in range(IMG):
            pbcast(tau_b[b][:], tau[b][NLEV - 1][:])
            v.tensor_scalar(s4[b][:], nv3[:, b, :], tau_b[b][:], 0.0, Alu.is_gt,
                            Alu.add, accum_out=cnt_pp[b][:])
            v.tensor_tensor(out=s5[b][:], in0=nv3[:, b, :], in1=s4[b][:],
                            op=Alu.mult)
            v.tensor_reduce(out=sum_pp[b][:], in_=s5[b][:], axis=Ax.X,
                            op=Alu.add)
            v.tensor_copy(stack[b][:, 0:1], npp[b][:])
            v.tensor_copy(stack[b][:, 1:2], locsum_pp[b][:])
            v.tensor_copy(stack[b][:, 2:3], possum_pp[b][:])
            v.tensor_copy(stack[b][:, 3:4], cnt_pp[b][:])
            st_ps[b] = pspool.tile([1, 4], F32, name=f"st_ps{b}", tag="pss")
            nc.tensor.matmul(st_ps[b][:], ones_col[:], stack[b][:])
            sm_ps[b] = pspool.tile([1, 1], F32, name=f"sm_ps{b}", tag="pss")
            nc.tensor.matmul(sm_ps[b][:], ones_col[:], sum_pp[b][:])
        for b in range(IMG):
            v.tensor_copy(res_sb[b][:, 0:4], st_ps[b][:])
            v.tensor_copy(res_sb[b][:, 4:5], sm_ps[b][:])
            v.tensor_copy(res_sb[b][:, 5:6], tau[b][NLEV - 1][:])
            v.tensor_copy(res_sb[b][:, 6:7], maxv1[b][:])
            v.tensor_copy(res_sb[b][:, 7:8], kk[b][:])
            nc.sync.dma_start(res_d[b], res_sb[b][:])

    nc.compile()
    return nc


_NC_CACHE = None


def _get_nc():
    global _NC_CACHE
    if _NC_CACHE is None:
        _NC_CACHE = _build_nc()
    return _NC_CACHE


def _make_in_maps(inputs):
    bbox_pred = np.asarray(inputs["bbox_pred"], dtype=np.float32)
    conf_pred = np.asarray(inputs["conf_pred"], dtype=np.float32)
    anchors = np.asarray(inputs["anchors"], dtype=np.float32)
    gt_boxes = np.asarray(inputs["gt_boxes"], dtype=np.float32)

    anch_h = np.ascontiguousarray(
        anchors.reshape(P, COLS, 4).transpose(2, 0, 1).astype(np.float16))
    in_maps = []
    for i in range(NCORE):
        bsl = slice(IMG * i, IMG * (i + 1))
        bb = bbox_pred[bsl].reshape(IMG, P, COLS, 4).transpose(0, 3, 1, 2)
        gt = gt_boxes[bsl]                       # [IMG, G, 4]
        gt_k = gt.transpose(0, 2, 1).reshape(IMG, 1, 4 * G)
        area = (gt[:, :, 2] - gt[:, :, 0]) * (gt[:, :, 3] - gt[:, :, 1])
        cx = gt[:, :, 0] + gt[:, :, 2]
        cy = gt[:, :, 1] + gt[:, :, 3]
        gt8 = np.concatenate([gt, area[..., None], cx[..., None], cy[..., None],
                              np.ones_like(area)[..., None]], axis=2)
        gtm = np.zeros((IMG, P, G), dtype=np.float32)
        for ci in range(4):
            gtm[:, ci::4, ci * 8:(ci + 1) * 8] = gt8
        in_maps.append({
            "anch": anch_h,
            "bbox": np.ascontiguousarray(bb.astype(np.float16)),
            "conf": np.ascontiguousarray(
                conf_pred[bsl].reshape(IMG, P, COLS).astype(np.float16)),
            "gtb": np.ascontiguousarray(gt_k.astype(np.float16)),
            "gtm": np.ascontiguousarray(gtm.astype(np.float16)),
        })
    return in_maps


def kernel(bbox_pred, conf_pred, anchors, gt_boxes):
    nc = _get_nc()
    in_maps = _make_in_maps(dict(bbox_pred=bbox_pred, conf_pred=conf_pred,
                                 anchors=anchors, gt_boxes=gt_boxes))
    out = run_bass_kernel_spmd(nc, in_maps, core_ids=list(range(NCORE)))

    loc_total = np.float32(0.0)
    conf_total = np.float32(0.0)
    npos_total = np.float32(0.0)
    for i in range(NCORE):
        res = out.results[i]["res"]  # [IMG, 1, 8]
        for b in range(IMG):
            npos, locsum, possum, cnt_gt, sum_gt, tau_hi, maxv, kdev = \
                [np.float32(x) for x in res[b, 0, :8]]
            k = np.float32(min(NEG_POS_RATIO * npos, A - npos))
            wl_last = np.float32(maxv / NBIN ** NLEV)
            rem = max(np.float32(0.0), np.float32(k - cnt_gt))
            neg = np.float32(sum_gt + rem * (tau_hi - wl_last * np.float32(0.5)))
            loc_total = np.float32(loc_total + locsum)
            conf_total = np.float32(conf_total + possum + neg)
            npos_total = np.float32(npos_total + npos)
    num_pos = np.float32(max(1.0, npos_total))
    loc_loss = np.float32(loc_total / num_pos)
    conf_loss = np.float32(conf_total / num_pos)
    return (np.float32(loc_loss + conf_loss), conf_loss, loc_loss)
